# revision 33
# baseline (speedup 1.0000x reference)
"""Causal self-attention (B=2, S=2048, D=2048, H=16, HD=128) on 8 TRN2 cores.

Sharding: core c -> batch b = c//4, heads 4*(c%4)..4*(c%4)+3 (tensor-parallel
over heads within a batch; data-parallel over batch across core groups).

v5 design (v3 ~400us, v4.5 ~337us at full clock):
  - RoPE via host-side de-interleave permutation of Wq/Wk columns (evens then
    odds per head) + permuted cos/sin tables with the rotate sign folded into
    sin. rotate_half becomes a partition-half swap (two ACT copies reading the
    projection PSUM at partition offset 64). No pmat matmul.
  - merged pipeline: block sb's projections interleave with block sb-1's
    attention windows (one-block lag satisfies all deps), so projection
    matmuls fill the attention stream's exp-latency bubbles and attention's
    ACT/DVE load spreads over the whole timeline. The PE clocks down after
    any idle and needs 3us of continuous execution to re-reach max speed, so
    continuous feed matters twice.
  - softmax row-sums off the PE: DVE accumulates exp chunks; one ones-matrix
    matmul per window produces a *replicated* [128,512] row-sum (M=128 costs
    the same as M=1) so the reciprocal (approx_fast, ~18 bits, output is
    bf16 anyway) needs no partition broadcast.
  - per-512-block q/k/ctx/v tiles: dependency tracking is tile-granular, so
    shared tiles would stall attention on unrelated later writes.
  - head-priority DMA: x0/wv eighths lead the scalar/sync rings; V-group
    projections run first so the RoPE tables can trail on the scalar ring.
  - outproj dripped as 4-matmul sub-jobs into window slots (with a held-back
    reserve for window starts); final drain alternates two PSUM pools.
"""

import math
from collections import deque

import ml_dtypes
import numpy as np

import concourse.bacc as bacc
import concourse.mybir as mybir
from concourse.tile import TileContext
from concourse.bass_utils import run_bass_kernel_spmd

B, S, D = 2, 2048, 2048
H, HD = 16, 128
ROPE_THETA = 10000.0

N_CORES = 8
CORES_PER_BATCH = 4
HPC = H // (N_CORES // B)  # heads per core = 4
HL = HPC * HD              # 512 local head-dim columns
NDC = D // 128             # 16 contraction chunks
NSB = S // 512             # 4 s-blocks
NKC = S // 128             # 16 k-chunks

F32 = mybir.dt.float32
BF16 = mybir.dt.bfloat16
AF = mybir.ActivationFunctionType
BNP = ml_dtypes.bfloat16

# projection finishers are emitted immediately after their group: they read
# the group's PSUM, and interleaved attention chunks allocate from the same
# PSUM ring — any lag lets the ring wrap and clobber the PSUM before the
# finisher's read is registered
FIN_LAG = 0
PV_LAG = 3    # PV matmuls lag score matmuls by this many chunks


def _mm(nc, out, lhsT, rhs, start, stop):
    nc.tensor.matmul(out, lhsT, rhs, start=start, stop=stop)


def _build():
    nc = bacc.Bacc("TRN2", target_bir_lowering=False, debug=False)

    # all big operands arrive pre-rearranged to the on-chip layout so each
    # DMA is one contiguous run per partition
    xT = nc.dram_tensor("xT", [128, NSB, NDC, 512], BF16, kind="ExternalInput")
    wq = nc.dram_tensor("wq", [128, NDC, HL], BF16, kind="ExternalInput")
    wk = nc.dram_tensor("wk", [128, NDC, HL], BF16, kind="ExternalInput")
    wv = nc.dram_tensor("wv", [128, NDC, HL], BF16, kind="ExternalInput")
    wo = nc.dram_tensor("wo", [128, HL // 128, D], BF16, kind="ExternalInput")
    cosP = nc.dram_tensor("cosP", [HD, S], BF16, kind="ExternalInput")
    sinP = nc.dram_tensor("sinP", [HD, S], BF16, kind="ExternalInput")
    maskT = nc.dram_tensor("maskT", [128, 512], BF16, kind="ExternalInput")
    onesd = nc.dram_tensor("onesd", [128, 128], BF16, kind="ExternalInput")
    out = nc.dram_tensor("out", [S, D], BF16, kind="ExternalOutput")

    with TileContext(nc) as tc:
        with (
            tc.tile_pool(name="consts", bufs=1) as consts,
            tc.tile_pool(name="resid", bufs=1) as resid,
            tc.tile_pool(name="wpool", bufs=1) as wpool,
            tc.tile_pool(name="xtp", bufs=2) as xtp,
            tc.tile_pool(name="st1", bufs=2) as st1,
            tc.tile_pool(name="pp", bufs=8) as pp,
            tc.tile_pool(name="accp", bufs=3) as accp,
            tc.tile_pool(name="sm", bufs=2) as sm,
            tc.tile_pool(name="pvp", bufs=3) as pvp,
            tc.tile_pool(name="outp", bufs=4) as outp,
            tc.tile_pool(name="psA", bufs=4, space="PSUM") as psA,
            tc.tile_pool(name="psB", bufs=2, space="PSUM") as psB,
            tc.tile_pool(name="psD", bufs=2, space="PSUM") as psD,
        ):
            # SBUF-resident q^T/k^T (per head, RoPE'd+permuted), v, ctx; one
            # tile per 512-block (dependency tracking is tile-granular)
            q_sb = [[resid.tile([HD, 512], BF16, name=f"qT{h}_{b}")
                     for b in range(NSB)] for h in range(HPC)]
            k_sb = [[resid.tile([HD, 512], BF16, name=f"kT{h}_{b}")
                     for b in range(NSB)] for h in range(HPC)]
            v_sb = [resid.tile([128, 4, HL], BF16, name=f"v_sb{b}")
                    for b in range(NSB)]
            ctxs = [[resid.tile([128, 512], BF16, name=f"ctxT{h}_{b}")
                     for b in range(NSB)] for h in range(HPC)]

            cos_sb = consts.tile([HD, S], BF16, name="cos_sb")
            sin_sb = consts.tile([HD, S], BF16, name="sin_sb")
            mask_sb = consts.tile([128, 512], BF16, name="mask_sb")
            ones_sb = consts.tile([128, 128], BF16, name="ones_sb")
            gpwarm = consts.tile([128, 128], F32, name="gpwarm")
            wo_sb = consts.tile([128, HPC * D], BF16, name="wo_sb")

            w_sb = {}
            for nm in ("wq", "wk", "wv"):
                w_sb[nm] = wpool.tile([128, NDC, HL], BF16, name=f"{nm}_sb")

            # ---------------- DMA ring priority ----------------
            # (only sync/scalar/gpsimd can issue DMAs; gpsimd's library load
            # stalls its queue ~11us at boot). sync: wv eighths, wq quarters,
            # wk, wo. scalar: x0 eighths, RoPE tables, x1. gpsimd: warmup,
            # then x2/x3.
            for qt in range(8):
                nc.sync.dma_start(
                    out=w_sb["wv"][:, qt * 2:(qt + 1) * 2, :],
                    in_=wv[:, qt * 2:(qt + 1) * 2, :])

            def load_x_block(sb, parts=1, eng=None):
                eng = eng or nc.scalar
                xt = xtp.tile([128, NDC, 512], BF16, tag="xt", name="xt")
                step = NDC // parts
                for qt in range(parts):
                    eng.dma_start(
                        out=xt[:, qt * step:(qt + 1) * step, :],
                        in_=xT[:, sb, qt * step:(qt + 1) * step, :])
                return xt

            x_cur = load_x_block(0, parts=8)
            for qt in range(4):
                nc.sync.dma_start(
                    out=w_sb["wq"][:, qt * 4:(qt + 1) * 4, :],
                    in_=wq[:, qt * 4:(qt + 1) * 4, :])
            nc.scalar.dma_start(out=cos_sb[:], in_=cosP[:])
            nc.scalar.dma_start(out=sin_sb[:], in_=sinP[:])
            nc.scalar.dma_start(out=mask_sb[:], in_=maskT[:])
            nc.scalar.dma_start(out=ones_sb[:], in_=onesd[:])
            nc.sync.dma_start(out=w_sb["wk"][:], in_=wk[:])
            nc.sync.dma_start(out=wo_sb[:], in_=wo[:])
            x_next = load_x_block(1)
            # warm the gpsimd tensor-op library off the critical path
            # (matches the finalize multiply's op family)
            nc.gpsimd.tensor_mul(gpwarm[:], gpwarm[:], gpwarm[:])

            # ---------------- projection machinery ----------------
            finishers = deque()

            def emit_finisher():
                kind, args = finishers.popleft()
                if kind == "qk":
                    ps, dst, sb_i = args
                    sl = slice(sb_i * 512, (sb_i + 1) * 512)
                    # rotate_half in permuted space = swap 64-halves; the
                    # swap copies run on ACT, the sign lives in sin_sb
                    # (host-folded)
                    qsw = st1.tile([128, 512], F32, tag="qsw", name="qsw")
                    nc.scalar.activation(qsw[0:64, :], ps[64:128, :], AF.Copy)
                    nc.scalar.activation(qsw[64:128, :], ps[0:64, :], AF.Copy)
                    ta = st1.tile([128, 512], F32, tag="ta", name="ta")
                    nc.vector.tensor_mul(ta[:], ps[:], cos_sb[:, sl])
                    tb = st1.tile([128, 512], F32, tag="tb", name="tb")
                    nc.vector.tensor_mul(tb[:], qsw[:], sin_sb[:, sl])
                    nc.vector.tensor_add(dst[:], ta[:], tb[:])
                else:
                    ps, kc = args
                    nc.scalar.activation(v_sb[kc // 4][:, kc % 4, :], ps[:],
                                         AF.Copy)

            def emit_group(sb, kind, idx):
                ps = psA.tile([128, 512], F32, tag="a", name="ps")
                if kind == "v":
                    for dc in range(NDC):
                        _mm(nc, ps[:],
                            x_cur[:, dc, idx * 128:(idx + 1) * 128],
                            w_sb["wv"][:, dc, :],
                            start=(dc == 0), stop=(dc == NDC - 1))
                    finishers.append(("v", (ps, sb * 4 + idx)))
                else:
                    w_t = w_sb["wq"] if kind == "q" else w_sb["wk"]
                    dst = (q_sb if kind == "q" else k_sb)[idx][sb]
                    for dc in range(NDC):
                        _mm(nc, ps[:],
                            w_t[:, dc, idx * HD:(idx + 1) * HD],
                            x_cur[:, dc, :],
                            start=(dc == 0), stop=(dc == NDC - 1))
                    finishers.append(("qk", (ps, dst, sb)))
                if len(finishers) > FIN_LAG:
                    emit_finisher()

            # ---------------- attention machinery ----------------
            lagq = deque()
            Oq = deque()          # outproj sub-jobs (qc, db)
            done_cnt = [0] * NSB  # finalized heads per q-block
            st = {"prev_w": None}

            def emit_lpv(job):
                pv, pt, vtc, ncols, first, last, w = job
                _mm(nc, pv[:, 512 - ncols:], vtc, pt[:, :ncols],
                    start=first, stop=last)
                if last:
                    # eager PSUM evacuation (ACT) so the pv bank recycles
                    pvs = pvp.tile([128, 512], BF16, tag="pvs", name="pvs")
                    nc.scalar.activation(pvs[:], pv[:], AF.Copy)
                    w["pvs"] = pvs

            def emit_chain_a(w):
                # replicated row-sum: ones[128,128]^T @ acc -> every
                # partition holds l, so no partition_broadcast needed.
                # lps shares psA's ring (held about as long as an sps tile).
                lps = psA.tile([128, 512], F32, tag="a", name="lps")
                _mm(nc, lps[:], ones_sb[:], w["acc"][:], start=True, stop=True)
                rcp = sm.tile([128, 512], F32, tag="rcp", name="rcp")
                nc.vector.reciprocal_approx_fast(rcp[:], lps[:])
                w["rcp"] = rcp

            def emit_chain_b(w, eng=None):
                qb = w["qb"]
                # SBUF-only operands, so this can live on gpsimd
                (eng or nc.gpsimd).tensor_mul(
                    ctxs[w["h"]][qb][:], w["pvs"][:], w["rcp"][:])
                done_cnt[qb] += 1
                if done_cnt[qb] == HPC:
                    for qc in range(4 * qb, 4 * qb + 4):
                        for db in range(D // 512):
                            Oq.append((qc, db))

            def emit_outproj(pool=None, tag="d"):
                qc, db = Oq.popleft()
                ops = (pool or psD).tile([128, 512], F32, tag=tag, name="ops")
                for hh in range(HPC):
                    _mm(nc, ops[:],
                        ctxs[hh][qc // 4][:, (qc % 4) * 128:(qc % 4 + 1) * 128],
                        wo_sb[:, hh * D + db * 512:hh * D + (db + 1) * 512],
                        start=(hh == 0), stop=(hh == HPC - 1))
                osb = outp.tile([128, 512], BF16, tag="osb", name="osb")
                # gpsimd cannot read PSUM; alternate evacuation ACT/DVE
                if (qc + db) % 2 == 0:
                    nc.scalar.activation(osb[:], ops[:], AF.Copy)
                else:
                    nc.vector.tensor_copy(osb[:], ops[:])
                nc.sync.dma_start(
                    out=out[qc * 128:(qc + 1) * 128, db * 512:(db + 1) * 512],
                    in_=osb[:])

            def window_stream():
                for qb in range(NSB):
                    for h in range(HPC):
                        nk = 4 * qb + 4
                        w = {"h": h, "qb": qb}
                        pv = psB.tile([128, 512], F32, tag="b", name="pv")
                        acc = accp.tile([128, 512], BF16, tag="acc",
                                        name="acc")
                        w["acc"] = acc
                        # diagonal (masked) chunks first: their 2 DVE ops per
                        # chunk overlap the previous window's light tail
                        order = (list(range(4 * qb, nk))
                                 + list(range(0, 4 * qb)))
                        for ci, kc in enumerate(order):
                            j = kc - 4 * qb
                            ncols = 512 if j < 0 else 512 - 128 * j
                            sps = psA.tile([128, 512], F32, tag="a",
                                           name="sps")
                            _mm(nc, sps[:, :ncols],
                                k_sb[h][kc // 4][:, (kc % 4) * 128:
                                                 (kc % 4 + 1) * 128],
                                q_sb[h][qb][:, 512 - ncols:],
                                start=True, stop=True)
                            pt = pp.tile([128, 512], BF16, tag="pt",
                                         name="pt")
                            nc.scalar.activation(pt[:, :ncols],
                                                 sps[:, :ncols], AF.Exp)
                            if j >= 0:
                                nc.vector.tensor_mul(pt[:, :ncols],
                                                     pt[:, :ncols],
                                                     mask_sb[:, :ncols])
                            if ci == 0:
                                nc.vector.tensor_copy(acc[:], pt[:])
                            else:
                                nc.vector.tensor_add(acc[:, 512 - ncols:],
                                                     acc[:, 512 - ncols:],
                                                     pt[:, :ncols])
                            lagq.append((pv, pt,
                                         v_sb[kc // 4][:, kc % 4,
                                                       h * HD:(h + 1) * HD],
                                         ncols, ci == 0, ci == nk - 1, w))
                            while len(lagq) > PV_LAG:
                                emit_lpv(lagq.popleft())
                            # one side action per chunk: window starts carry
                            # outproj fill; the l-chain runs late enough to
                            # never wait on the accumulation engines; late
                            # slots hold back a reserve for the next start
                            ca, cb = (2, 3) if nk <= 4 else (4, 5)
                            prev_w = st["prev_w"]
                            if ci == ca and prev_w is not None:
                                emit_chain_a(prev_w)
                            elif ci == cb and prev_w is not None:
                                emit_chain_b(prev_w)
                            elif Oq and (ci < ca or len(Oq) > 4):
                                emit_outproj()
                            yield
                        st["prev_w"] = w

            ws = window_stream()

            def pump(n):
                for _ in range(n):
                    next(ws, None)

            # ---------------- merged emission ----------------
            # block sb emits sb's 12 projection groups interleaved with the
            # chunks of block sb-1's attention windows (16*sb chunks)
            for sb in range(NSB):
                if sb > 0:
                    x_cur = x_next
                    if sb < NSB - 1:
                        x_next = load_x_block(sb + 1, eng=nc.gpsimd)
                groups = ([("v", i) for i in range(4)]
                          + [("q", h) for h in range(HPC)]
                          + [("k", h) for h in range(HPC)])
                nchunks = 16 * sb
                done = 0
                for gi, (kind, idx) in enumerate(groups):
                    emit_group(sb, kind, idx)
                    share = nchunks * (gi + 1) // len(groups) - done
                    pump(share)
                    done += share
                while finishers:
                    emit_finisher()

            # remaining windows (qb = NSB-1) run un-interleaved
            _SENT = object()
            while next(ws, _SENT) is not _SENT:
                pass
            while lagq:
                emit_lpv(lagq.popleft())
            emit_chain_a(st["prev_w"])
            emit_chain_b(st["prev_w"], eng=nc.vector)
            # final outproj drain: alternate two PSUM pools so the next
            # job's matmuls never wait on the previous evacuation
            flip = 0
            while Oq:
                if flip % 2 == 0:
                    emit_outproj(pool=psD, tag="d")
                else:
                    emit_outproj(pool=psA, tag="a")
                flip += 1

    nc.compile()
    return nc


_NC_CACHE = None


def _get_nc():
    global _NC_CACHE
    if _NC_CACHE is None:
        _NC_CACHE = _build()
    return _NC_CACHE


# de-interleave: evens then odds, per head
_PERM = np.concatenate([np.arange(0, HD, 2), np.arange(1, HD, 2)])


def _host_tables():
    # Replicate reference RoPE tables in float32 arithmetic, permuted.
    inv_freq = np.float32(1.0) / np.power(
        np.float32(ROPE_THETA), np.arange(0, HD, 2).astype(np.float32) / np.float32(HD)
    )
    pos = np.arange(S, dtype=np.float32)
    freqs = pos[:, None] * inv_freq[None, :]
    angles = np.concatenate([freqs, freqs], axis=1)  # [S, HD]
    cos = np.cos(angles).astype(np.float32)
    sin = np.sin(angles).astype(np.float32)
    cos_p = np.ascontiguousarray(cos[:, _PERM].T)  # [HD, S]
    sin_p = np.ascontiguousarray(sin[:, _PERM].T).copy()
    sin_p[:HD // 2] *= np.float32(-1.0)  # fold rotate_half's sign
    mask = (np.arange(128)[:, None] <= np.arange(512)[None, :]).astype(BNP)
    return cos_p.astype(BNP), sin_p.astype(BNP), mask


_ONES = np.ones((128, 128), dtype=BNP)


def kernel(x, Wq, Wk, Wv, Wo):
    x = np.asarray(x, dtype=np.float32)
    Wq = np.asarray(Wq, dtype=np.float32)
    Wk = np.asarray(Wk, dtype=np.float32)
    Wv = np.asarray(Wv, dtype=np.float32)
    Wo = np.asarray(Wo, dtype=np.float32)

    results = _run_device(x, Wq, Wk, Wv, Wo)

    out = np.empty((B, S, D), dtype=np.float32)
    for b in range(B):
        acc = results[b * CORES_PER_BATCH]["out"].astype(np.float32)
        for i in range(1, CORES_PER_BATCH):
            acc = acc + results[b * CORES_PER_BATCH + i]["out"].astype(np.float32)
        out[b] = acc
    return out


def _make_in_maps(x, Wq, Wk, Wv, Wo):
    cos_p, sin_p, mask = _host_tables()
    scale = np.float32(1.0 / math.sqrt(HD))
    # permutation of a 512-col (4-head) slice: de-interleave within each head
    block_perm = np.concatenate([hh * HD + _PERM for hh in range(HPC)])

    def dev_w(w):  # [D, HL_slice] -> [128, NDC, hl]
        return np.ascontiguousarray(
            w.reshape(NDC, 128, -1).transpose(1, 0, 2)).astype(BNP)

    wq_scaled = (Wq * scale).astype(np.float32)
    xTb = [
        np.ascontiguousarray(
            x[b].T.reshape(NDC, 128, NSB, 512).transpose(1, 2, 0, 3)).astype(BNP)
        for b in range(B)
    ]
    in_maps = []
    for c in range(N_CORES):
        b = c // CORES_PER_BATCH
        g = c % CORES_PER_BATCH
        hs = slice(g * HL, (g + 1) * HL)
        in_maps.append({
            "xT": xTb[b],
            "wq": dev_w(wq_scaled[:, hs][:, block_perm]),
            "wk": dev_w(Wk[:, hs][:, block_perm]),
            "wv": dev_w(Wv[:, hs]),
            "wo": np.ascontiguousarray(
                Wo[hs, :].reshape(HL // 128, 128, D).transpose(1, 0, 2)).astype(BNP),
            "cosP": cos_p,
            "sinP": sin_p,
            "maskT": mask,
            "onesd": _ONES,
        })
    return in_maps


def _run_device(x, Wq, Wk, Wv, Wo, trace=False):
    nc = _get_nc()
    in_maps = _make_in_maps(x, Wq, Wk, Wv, Wo)
    res = run_bass_kernel_spmd(nc, in_maps, core_ids=list(range(N_CORES)), trace=trace)
    if trace:
        return res
    return res.results


def run_traced(x, Wq, Wk, Wv, Wo):
    """Run with NTFF tracing; returns (full_output, BassKernelResults)."""
    res = _run_device(np.asarray(x, np.float32), np.asarray(Wq, np.float32),
                      np.asarray(Wk, np.float32), np.asarray(Wv, np.float32),
                      np.asarray(Wo, np.float32), trace=True)
    out = np.empty((B, S, D), dtype=np.float32)
    for b in range(B):
        acc = res.results[b * CORES_PER_BATCH]["out"].astype(np.float32)
        for i in range(1, CORES_PER_BATCH):
            acc = acc + res.results[b * CORES_PER_BATCH + i]["out"].astype(np.float32)
        out[b] = acc
    return out, res


# revision 35
# speedup vs baseline: 1.0078x; 1.0078x over previous
"""Causal self-attention (B=2, S=2048, D=2048, H=16, HD=128) on 8 TRN2 cores.

Sharding: core c -> batch b = c//4, heads 4*(c%4)..4*(c%4)+3 (tensor-parallel
over heads within a batch; data-parallel over batch across core groups).

v5 design (v3 ~400us, v4.5 ~337us at full clock):
  - RoPE via host-side de-interleave permutation of Wq/Wk columns (evens then
    odds per head) + permuted cos/sin tables with the rotate sign folded into
    sin. rotate_half becomes a partition-half swap (two ACT copies reading the
    projection PSUM at partition offset 64). No pmat matmul.
  - merged pipeline: block sb's projections interleave with block sb-1's
    attention windows (one-block lag satisfies all deps), so projection
    matmuls fill the attention stream's exp-latency bubbles and attention's
    ACT/DVE load spreads over the whole timeline. The PE clocks down after
    any idle and needs 3us of continuous execution to re-reach max speed, so
    continuous feed matters twice.
  - softmax row-sums off the PE: DVE accumulates exp chunks; one ones-matrix
    matmul per window produces a *replicated* [128,512] row-sum (M=128 costs
    the same as M=1) so the reciprocal (approx_fast, ~18 bits, output is
    bf16 anyway) needs no partition broadcast.
  - per-512-block q/k/ctx/v tiles: dependency tracking is tile-granular, so
    shared tiles would stall attention on unrelated later writes.
  - head-priority DMA: x0/wv eighths lead the scalar/sync rings; V-group
    projections run first so the RoPE tables can trail on the scalar ring.
  - outproj dripped as 4-matmul sub-jobs into window slots (with a held-back
    reserve for window starts); final drain alternates two PSUM pools.
"""

import math
from collections import deque

import ml_dtypes
import numpy as np

import concourse.bacc as bacc
import concourse.mybir as mybir
from concourse.tile import TileContext
from concourse.bass_utils import run_bass_kernel_spmd

B, S, D = 2, 2048, 2048
H, HD = 16, 128
ROPE_THETA = 10000.0

N_CORES = 8
CORES_PER_BATCH = 4
HPC = H // (N_CORES // B)  # heads per core = 4
HL = HPC * HD              # 512 local head-dim columns
NDC = D // 128             # 16 contraction chunks
NSB = S // 512             # 4 s-blocks
NKC = S // 128             # 16 k-chunks

F32 = mybir.dt.float32
BF16 = mybir.dt.bfloat16
AF = mybir.ActivationFunctionType
BNP = ml_dtypes.bfloat16

# projection finishers are emitted immediately after their group: they read
# the group's PSUM, and interleaved attention chunks allocate from the same
# PSUM ring — any lag lets the ring wrap and clobber the PSUM before the
# finisher's read is registered
FIN_LAG = 0
PV_LAG = 3    # PV matmuls lag score matmuls by this many chunks


def _mm(nc, out, lhsT, rhs, start, stop):
    nc.tensor.matmul(out, lhsT, rhs, start=start, stop=stop)


def _build():
    nc = bacc.Bacc("TRN2", target_bir_lowering=False, debug=False)

    # all big operands arrive pre-rearranged to the on-chip layout so each
    # DMA is one contiguous run per partition
    xT = nc.dram_tensor("xT", [128, NSB, NDC, 512], BF16, kind="ExternalInput")
    wq = nc.dram_tensor("wq", [128, NDC, HL], BF16, kind="ExternalInput")
    wk = nc.dram_tensor("wk", [128, NDC, HL], BF16, kind="ExternalInput")
    wv = nc.dram_tensor("wv", [128, NDC, HL], BF16, kind="ExternalInput")
    wo = nc.dram_tensor("wo", [128, HL // 128, D], BF16, kind="ExternalInput")
    cosP = nc.dram_tensor("cosP", [HD, S], BF16, kind="ExternalInput")
    sinP = nc.dram_tensor("sinP", [HD, S], BF16, kind="ExternalInput")
    maskT = nc.dram_tensor("maskT", [128, 512], BF16, kind="ExternalInput")
    onesd = nc.dram_tensor("onesd", [128, 128], BF16, kind="ExternalInput")
    out = nc.dram_tensor("out", [S, D], BF16, kind="ExternalOutput")

    with TileContext(nc) as tc:
        with (
            tc.tile_pool(name="consts", bufs=1) as consts,
            tc.tile_pool(name="resid", bufs=1) as resid,
            tc.tile_pool(name="wpool", bufs=1) as wpool,
            tc.tile_pool(name="xtp", bufs=2) as xtp,
            tc.tile_pool(name="st1", bufs=2) as st1,
            tc.tile_pool(name="pp", bufs=8) as pp,
            tc.tile_pool(name="accp", bufs=3) as accp,
            tc.tile_pool(name="sm", bufs=2) as sm,
            tc.tile_pool(name="pvp", bufs=3) as pvp,
            tc.tile_pool(name="outp", bufs=4) as outp,
            tc.tile_pool(name="psA", bufs=4, space="PSUM") as psA,
            tc.tile_pool(name="psB", bufs=2, space="PSUM") as psB,
            tc.tile_pool(name="psD", bufs=2, space="PSUM") as psD,
        ):
            # SBUF-resident q^T/k^T (per head, RoPE'd+permuted), v, ctx; one
            # tile per 512-block (dependency tracking is tile-granular)
            q_sb = [[resid.tile([HD, 512], BF16, name=f"qT{h}_{b}")
                     for b in range(NSB)] for h in range(HPC)]
            k_sb = [[resid.tile([HD, 512], BF16, name=f"kT{h}_{b}")
                     for b in range(NSB)] for h in range(HPC)]
            v_sb = [resid.tile([128, 4, HL], BF16, name=f"v_sb{b}")
                    for b in range(NSB)]
            ctxs = [[resid.tile([128, 512], BF16, name=f"ctxT{h}_{b}")
                     for b in range(NSB)] for h in range(HPC)]

            cos_sb = consts.tile([HD, S], BF16, name="cos_sb")
            sin_sb = consts.tile([HD, S], BF16, name="sin_sb")
            mask_sb = consts.tile([128, 512], BF16, name="mask_sb")
            ones_sb = consts.tile([128, 128], BF16, name="ones_sb")
            gpwarm = consts.tile([128, 128], F32, name="gpwarm")
            wo_sb = consts.tile([128, HPC * D], BF16, name="wo_sb")

            w_sb = {}
            for nm in ("wq", "wk", "wv"):
                w_sb[nm] = wpool.tile([128, NDC, HL], BF16, name=f"{nm}_sb")

            # ---------------- DMA ring priority ----------------
            # (only sync/scalar/gpsimd can issue DMAs; gpsimd's library load
            # stalls its queue ~11us at boot). sync: wv eighths, wq quarters,
            # wk, wo. scalar: x0 eighths, RoPE tables, x1. gpsimd: warmup,
            # then x2/x3.
            for qt in range(8):
                nc.sync.dma_start(
                    out=w_sb["wv"][:, qt * 2:(qt + 1) * 2, :],
                    in_=wv[:, qt * 2:(qt + 1) * 2, :])

            def load_x_block(sb, parts=1, eng=None):
                eng = eng or nc.scalar
                xt = xtp.tile([128, NDC, 512], BF16, tag="xt", name="xt")
                step = NDC // parts
                for qt in range(parts):
                    eng.dma_start(
                        out=xt[:, qt * step:(qt + 1) * step, :],
                        in_=xT[:, sb, qt * step:(qt + 1) * step, :])
                return xt

            x_cur = load_x_block(0, parts=8)
            for qt in range(4):
                nc.sync.dma_start(
                    out=w_sb["wq"][:, qt * 4:(qt + 1) * 4, :],
                    in_=wq[:, qt * 4:(qt + 1) * 4, :])
            nc.scalar.dma_start(out=cos_sb[:], in_=cosP[:])
            nc.scalar.dma_start(out=sin_sb[:], in_=sinP[:])
            nc.scalar.dma_start(out=mask_sb[:], in_=maskT[:])
            nc.scalar.dma_start(out=ones_sb[:], in_=onesd[:])
            nc.sync.dma_start(out=wo_sb[:], in_=wo[:])
            x_next = load_x_block(1)
            # warm the gpsimd tensor-op library off the critical path
            # (matches the finalize multiply's op family), then wk rides the
            # gpsimd ring so it doesn't queue behind wq on sync
            nc.gpsimd.tensor_mul(gpwarm[:], gpwarm[:], gpwarm[:])
            nc.gpsimd.dma_start(out=w_sb["wk"][:], in_=wk[:])

            # ---------------- projection machinery ----------------
            finishers = deque()

            def emit_finisher():
                kind, args = finishers.popleft()
                if kind == "qk":
                    ps, dst, sb_i = args
                    sl = slice(sb_i * 512, (sb_i + 1) * 512)
                    # rotate_half in permuted space = swap 64-halves; the
                    # swap copies run on ACT, the sign lives in sin_sb
                    # (host-folded)
                    qsw = st1.tile([128, 512], F32, tag="qsw", name="qsw")
                    nc.scalar.activation(qsw[0:64, :], ps[64:128, :], AF.Copy)
                    nc.scalar.activation(qsw[64:128, :], ps[0:64, :], AF.Copy)
                    ta = st1.tile([128, 512], F32, tag="ta", name="ta")
                    nc.vector.tensor_mul(ta[:], ps[:], cos_sb[:, sl])
                    tb = st1.tile([128, 512], F32, tag="tb", name="tb")
                    nc.vector.tensor_mul(tb[:], qsw[:], sin_sb[:, sl])
                    nc.vector.tensor_add(dst[:], ta[:], tb[:])
                else:
                    ps, kc = args
                    nc.scalar.activation(v_sb[kc // 4][:, kc % 4, :], ps[:],
                                         AF.Copy)

            def emit_group(sb, kind, idx):
                ps = psA.tile([128, 512], F32, tag="a", name="ps")
                if kind == "v":
                    for dc in range(NDC):
                        _mm(nc, ps[:],
                            x_cur[:, dc, idx * 128:(idx + 1) * 128],
                            w_sb["wv"][:, dc, :],
                            start=(dc == 0), stop=(dc == NDC - 1))
                    finishers.append(("v", (ps, sb * 4 + idx)))
                else:
                    w_t = w_sb["wq"] if kind == "q" else w_sb["wk"]
                    dst = (q_sb if kind == "q" else k_sb)[idx][sb]
                    for dc in range(NDC):
                        _mm(nc, ps[:],
                            w_t[:, dc, idx * HD:(idx + 1) * HD],
                            x_cur[:, dc, :],
                            start=(dc == 0), stop=(dc == NDC - 1))
                    finishers.append(("qk", (ps, dst, sb)))
                if len(finishers) > FIN_LAG:
                    emit_finisher()

            # ---------------- attention machinery ----------------
            lagq = deque()
            Oq = deque()          # outproj sub-jobs (qc, db)
            done_cnt = [0] * NSB  # finalized heads per q-block
            st = {"prev_w": None}

            def emit_lpv(job):
                pv, pt, vtc, ncols, first, last, w = job
                _mm(nc, pv[:, 512 - ncols:], vtc, pt[:, :ncols],
                    start=first, stop=last)
                if last:
                    # eager PSUM evacuation (ACT) so the pv bank recycles
                    pvs = pvp.tile([128, 512], BF16, tag="pvs", name="pvs")
                    nc.scalar.activation(pvs[:], pv[:], AF.Copy)
                    w["pvs"] = pvs

            def emit_chain_a(w):
                # replicated row-sum: ones[128,128]^T @ acc -> every
                # partition holds l, so no partition_broadcast needed.
                # lps shares psA's ring (held about as long as an sps tile).
                lps = psA.tile([128, 512], F32, tag="a", name="lps")
                _mm(nc, lps[:], ones_sb[:], w["acc"][:], start=True, stop=True)
                rcp = sm.tile([128, 512], F32, tag="rcp", name="rcp")
                nc.vector.reciprocal_approx_fast(rcp[:], lps[:])
                w["rcp"] = rcp

            def emit_chain_b(w, eng=None):
                qb = w["qb"]
                # SBUF-only operands, so this can live on gpsimd
                (eng or nc.gpsimd).tensor_mul(
                    ctxs[w["h"]][qb][:], w["pvs"][:], w["rcp"][:])
                done_cnt[qb] += 1
                if done_cnt[qb] == HPC:
                    for qc in range(4 * qb, 4 * qb + 4):
                        for db in range(D // 512):
                            Oq.append((qc, db))

            def emit_outproj(pool=None, tag="d"):
                qc, db = Oq.popleft()
                ops = (pool or psD).tile([128, 512], F32, tag=tag, name="ops")
                for hh in range(HPC):
                    _mm(nc, ops[:],
                        ctxs[hh][qc // 4][:, (qc % 4) * 128:(qc % 4 + 1) * 128],
                        wo_sb[:, hh * D + db * 512:hh * D + (db + 1) * 512],
                        start=(hh == 0), stop=(hh == HPC - 1))
                osb = outp.tile([128, 512], BF16, tag="osb", name="osb")
                # gpsimd cannot read PSUM; alternate evacuation ACT/DVE
                if (qc + db) % 2 == 0:
                    nc.scalar.activation(osb[:], ops[:], AF.Copy)
                else:
                    nc.vector.tensor_copy(osb[:], ops[:])
                nc.sync.dma_start(
                    out=out[qc * 128:(qc + 1) * 128, db * 512:(db + 1) * 512],
                    in_=osb[:])

            def window_stream():
                for qb in range(NSB):
                    for h in range(HPC):
                        nk = 4 * qb + 4
                        w = {"h": h, "qb": qb}
                        pv = psB.tile([128, 512], F32, tag="b", name="pv")
                        acc = accp.tile([128, 512], BF16, tag="acc",
                                        name="acc")
                        w["acc"] = acc
                        # diagonal (masked) chunks first: their 2 DVE ops per
                        # chunk overlap the previous window's light tail
                        order = (list(range(4 * qb, nk))
                                 + list(range(0, 4 * qb)))
                        for ci, kc in enumerate(order):
                            j = kc - 4 * qb
                            ncols = 512 if j < 0 else 512 - 128 * j
                            sps = psA.tile([128, 512], F32, tag="a",
                                           name="sps")
                            _mm(nc, sps[:, :ncols],
                                k_sb[h][kc // 4][:, (kc % 4) * 128:
                                                 (kc % 4 + 1) * 128],
                                q_sb[h][qb][:, 512 - ncols:],
                                start=True, stop=True)
                            pt = pp.tile([128, 512], BF16, tag="pt",
                                         name="pt")
                            nc.scalar.activation(pt[:, :ncols],
                                                 sps[:, :ncols], AF.Exp)
                            if j >= 0:
                                nc.vector.tensor_mul(pt[:, :ncols],
                                                     pt[:, :ncols],
                                                     mask_sb[:, :ncols])
                            if ci == 0:
                                nc.vector.tensor_copy(acc[:], pt[:])
                            else:
                                nc.vector.tensor_add(acc[:, 512 - ncols:],
                                                     acc[:, 512 - ncols:],
                                                     pt[:, :ncols])
                            lagq.append((pv, pt,
                                         v_sb[kc // 4][:, kc % 4,
                                                       h * HD:(h + 1) * HD],
                                         ncols, ci == 0, ci == nk - 1, w))
                            while len(lagq) > PV_LAG:
                                emit_lpv(lagq.popleft())
                            # one side action per chunk: window starts carry
                            # outproj fill; the l-chain runs late enough to
                            # never wait on the accumulation engines; late
                            # slots hold back a reserve for the next start
                            ca, cb = (2, 3) if nk <= 4 else (4, 5)
                            prev_w = st["prev_w"]
                            if ci == ca and prev_w is not None:
                                emit_chain_a(prev_w)
                            elif ci == cb and prev_w is not None:
                                emit_chain_b(prev_w)
                            elif Oq and (ci < ca or len(Oq) > 6):
                                emit_outproj()
                            yield
                        st["prev_w"] = w

            ws = window_stream()

            def pump(n):
                for _ in range(n):
                    next(ws, None)

            # ---------------- merged emission ----------------
            # block sb emits sb's 12 projection groups interleaved with the
            # chunks of block sb-1's attention windows (16*sb chunks)
            for sb in range(NSB):
                if sb > 0:
                    x_cur = x_next
                    if sb < NSB - 1:
                        x_next = load_x_block(sb + 1, eng=nc.gpsimd)
                groups = ([("v", i) for i in range(4)]
                          + [("q", h) for h in range(HPC)]
                          + [("k", h) for h in range(HPC)])
                nchunks = 16 * sb
                done = 0
                for gi, (kind, idx) in enumerate(groups):
                    emit_group(sb, kind, idx)
                    share = nchunks * (gi + 1) // len(groups) - done
                    pump(share)
                    done += share
                while finishers:
                    emit_finisher()

            # remaining windows (qb = NSB-1) run un-interleaved
            _SENT = object()
            while next(ws, _SENT) is not _SENT:
                pass
            while lagq:
                emit_lpv(lagq.popleft())
            emit_chain_a(st["prev_w"])
            emit_chain_b(st["prev_w"], eng=nc.vector)
            # final outproj drain: alternate two PSUM pools so the next
            # job's matmuls never wait on the previous evacuation
            flip = 0
            while Oq:
                if flip % 2 == 0:
                    emit_outproj(pool=psD, tag="d")
                else:
                    emit_outproj(pool=psA, tag="a")
                flip += 1

    nc.compile()
    return nc


_NC_CACHE = None


def _get_nc():
    global _NC_CACHE
    if _NC_CACHE is None:
        _NC_CACHE = _build()
    return _NC_CACHE


# de-interleave: evens then odds, per head
_PERM = np.concatenate([np.arange(0, HD, 2), np.arange(1, HD, 2)])


def _host_tables():
    # Replicate reference RoPE tables in float32 arithmetic, permuted.
    inv_freq = np.float32(1.0) / np.power(
        np.float32(ROPE_THETA), np.arange(0, HD, 2).astype(np.float32) / np.float32(HD)
    )
    pos = np.arange(S, dtype=np.float32)
    freqs = pos[:, None] * inv_freq[None, :]
    angles = np.concatenate([freqs, freqs], axis=1)  # [S, HD]
    cos = np.cos(angles).astype(np.float32)
    sin = np.sin(angles).astype(np.float32)
    cos_p = np.ascontiguousarray(cos[:, _PERM].T)  # [HD, S]
    sin_p = np.ascontiguousarray(sin[:, _PERM].T).copy()
    sin_p[:HD // 2] *= np.float32(-1.0)  # fold rotate_half's sign
    mask = (np.arange(128)[:, None] <= np.arange(512)[None, :]).astype(BNP)
    return cos_p.astype(BNP), sin_p.astype(BNP), mask


_ONES = np.ones((128, 128), dtype=BNP)


def kernel(x, Wq, Wk, Wv, Wo):
    x = np.asarray(x, dtype=np.float32)
    Wq = np.asarray(Wq, dtype=np.float32)
    Wk = np.asarray(Wk, dtype=np.float32)
    Wv = np.asarray(Wv, dtype=np.float32)
    Wo = np.asarray(Wo, dtype=np.float32)

    results = _run_device(x, Wq, Wk, Wv, Wo)

    out = np.empty((B, S, D), dtype=np.float32)
    for b in range(B):
        acc = results[b * CORES_PER_BATCH]["out"].astype(np.float32)
        for i in range(1, CORES_PER_BATCH):
            acc = acc + results[b * CORES_PER_BATCH + i]["out"].astype(np.float32)
        out[b] = acc
    return out


def _make_in_maps(x, Wq, Wk, Wv, Wo):
    cos_p, sin_p, mask = _host_tables()
    scale = np.float32(1.0 / math.sqrt(HD))
    # permutation of a 512-col (4-head) slice: de-interleave within each head
    block_perm = np.concatenate([hh * HD + _PERM for hh in range(HPC)])

    def dev_w(w):  # [D, HL_slice] -> [128, NDC, hl]
        return np.ascontiguousarray(
            w.reshape(NDC, 128, -1).transpose(1, 0, 2)).astype(BNP)

    wq_scaled = (Wq * scale).astype(np.float32)
    xTb = [
        np.ascontiguousarray(
            x[b].T.reshape(NDC, 128, NSB, 512).transpose(1, 2, 0, 3)).astype(BNP)
        for b in range(B)
    ]
    in_maps = []
    for c in range(N_CORES):
        b = c // CORES_PER_BATCH
        g = c % CORES_PER_BATCH
        hs = slice(g * HL, (g + 1) * HL)
        in_maps.append({
            "xT": xTb[b],
            "wq": dev_w(wq_scaled[:, hs][:, block_perm]),
            "wk": dev_w(Wk[:, hs][:, block_perm]),
            "wv": dev_w(Wv[:, hs]),
            "wo": np.ascontiguousarray(
                Wo[hs, :].reshape(HL // 128, 128, D).transpose(1, 0, 2)).astype(BNP),
            "cosP": cos_p,
            "sinP": sin_p,
            "maskT": mask,
            "onesd": _ONES,
        })
    return in_maps


def _run_device(x, Wq, Wk, Wv, Wo, trace=False):
    nc = _get_nc()
    in_maps = _make_in_maps(x, Wq, Wk, Wv, Wo)
    res = run_bass_kernel_spmd(nc, in_maps, core_ids=list(range(N_CORES)), trace=trace)
    if trace:
        return res
    return res.results


def run_traced(x, Wq, Wk, Wv, Wo):
    """Run with NTFF tracing; returns (full_output, BassKernelResults)."""
    res = _run_device(np.asarray(x, np.float32), np.asarray(Wq, np.float32),
                      np.asarray(Wk, np.float32), np.asarray(Wv, np.float32),
                      np.asarray(Wo, np.float32), trace=True)
    out = np.empty((B, S, D), dtype=np.float32)
    for b in range(B):
        acc = res.results[b * CORES_PER_BATCH]["out"].astype(np.float32)
        for i in range(1, CORES_PER_BATCH):
            acc = acc + res.results[b * CORES_PER_BATCH + i]["out"].astype(np.float32)
        out[b] = acc
    return out, res
